# revision 1
# baseline (speedup 1.0000x reference)
"""ChebNet (K=3, 2 layers) forward on 8 Trainium2 NeuronCores.

Self-contained: hardcodes the problem shapes (50000 nodes, 800000 edges,
128-d input, 128-d hidden, 40 classes).

Math: with sym-normalized Laplacian (lambda_max=2), the reference computes
  w_e = -dinv[src_e] * dinv[dst_e]           (separable!)
  spmv(h)[i] = sum_{dst_e=i} w_e * h[src_e]
so spmv(h) = -dinv ⊙ segsum(gather(dinv ⊙ h)).  All gathers read from
tables T = dinv ⊙ (recurrence vector), and the minus signs / factors of 2
in the Chebyshev recurrence are folded into premixed weight matrices:
  u1 = dinv⊙segsum(T0[src]), T0 = dinv⊙x        -> Tx1 = -u1
  v  = dinv⊙segsum(T1[src]), T1 = dinv⊙u1       -> Tx2 = 2v - x
  out1 = x@(W10-W12) + u1@(-W11) + v@(2 W12) + b1 ; h = relu(out1)
layer 2 identically with (p, q) and W2.

Per core: nodes [c*6250,(c+1)*6250), padded to 6272 = 49*128.  Edges are
bucketed by dst group (128 nodes) on host; per group the incoming edges are
split into lo/hi halves of the padded 50176-row table (int16 gather index
limit), each padded to a uniform tile count.  Segment-sum = one-hot matmuls
accumulated in PSUM (no HBM scatter).  Cross-core: AllGather of each table.
"""
import sys
sys.path.insert(0, '/opt/trn_rl_repo')
sys.path.insert(0, '/opt/trn_rl_repo/concourse')

import numpy as np

N_NODES = 50000
N_EDGES = 800000
D = 128
NCLS = 40
NCORES = 8
P = 128
CH = 2                            # dst groups per gather chunk


def _derive():
    global NLOC, NG, NLOCP, VPAD, HALF
    NLOC = N_NODES // NCORES          # 6250
    NG = (NLOC + P - 1) // P          # 49
    NLOCP = NG * P                    # 6272
    VPAD = NCORES * NLOCP             # 50176
    HALF = VPAD // 2                  # 25088


_derive()

_prog_cache = {}


def _build_program(TLO, THI, debug=False):
    import concourse.bacc as bacc
    import concourse.mybir as mybir
    import concourse.tile as tile
    from concourse.masks import make_identity
    from concourse.bass import _add_dep_helper

    f32 = mybir.dt.float32
    i16 = mybir.dt.int16
    Act = mybir.ActivationFunctionType
    Alu = mybir.AluOpType

    nc = bacc.Bacc(num_devices=NCORES, debug=debug,
                   dynamic_dma_scratch_size=32768)

    # ---- parameters (per-core values supplied via in_maps) ----
    x_in = nc.declare_dram_parameter("x", [NLOCP, D], f32, isOutput=False)
    gidx_lo_in = nc.declare_dram_parameter("gidx_lo", [128, NG * TLO * 8], i16, isOutput=False)
    gidx_hi_in = nc.declare_dram_parameter("gidx_hi", [128, NG * THI * 8], i16, isOutput=False)
    dstl_lo_in = nc.declare_dram_parameter("dstl_lo", [128, NG * TLO], i16, isOutput=False)
    dstl_hi_in = nc.declare_dram_parameter("dstl_hi", [128, NG * THI], i16, isOutput=False)
    dinv_loc_in = nc.declare_dram_parameter("dinv_loc", [128, NG], f32, isOutput=False)
    dinvb_in = nc.declare_dram_parameter("dinvb", [128, NLOCP], f32, isOutput=False)
    Acat_in = nc.declare_dram_parameter("Acat", [128, 3 * D], f32, isOutput=False)
    Bcat_in = nc.declare_dram_parameter("Bcat", [128, 3 * NCLS], f32, isOutput=False)
    b1_in = nc.declare_dram_parameter("b1", [128, 1], f32, isOutput=False)
    b2_in = nc.declare_dram_parameter("b2", [128, 1], f32, isOutput=False)
    out_par = nc.declare_dram_parameter("out", [NLOC, NCLS], f32, isOutput=True)

    # ---- internal DRAM: table slices + allgathered tables ----
    tslices = [nc.dram_tensor(f"t{i}s", [NLOCP, D], f32) for i in range(4)]
    tables = [nc.dram_tensor(f"T{i}", [VPAD, D], f32, addr_space="Shared")
              for i in range(4)]

    rg = [list(range(NCORES))]

    with tile.TileContext(nc) as tc:
        with tc.tile_pool(name="const", bufs=1) as cpool, \
             tc.tile_pool(name="big", bufs=3) as bigpool, \
             tc.tile_pool(name="gbuf", bufs=2) as gpool, \
             tc.tile_pool(name="work", bufs=3) as wpool, \
             tc.tile_pool(name="pacc", bufs=2, space="PSUM") as pacc, \
             tc.tile_pool(name="pmisc", bufs=2, space="PSUM") as pmisc:

            # ---------- constants ----------
            iota_t = cpool.tile([P, P], i16)
            nc.gpsimd.iota(iota_t[:], pattern=[[1, P]], base=0, channel_multiplier=0)
            ident = cpool.tile([P, P], f32)
            make_identity(nc, ident[:])
            gidx_lo = cpool.tile([128, NG * TLO * 8], i16)
            nc.sync.dma_start(out=gidx_lo[:], in_=gidx_lo_in[:])
            gidx_hi = cpool.tile([128, NG * THI * 8], i16)
            nc.sync.dma_start(out=gidx_hi[:], in_=gidx_hi_in[:])
            dstl_lo = cpool.tile([128, NG * TLO], i16)
            nc.sync.dma_start(out=dstl_lo[:], in_=dstl_lo_in[:])
            dstl_hi = cpool.tile([128, NG * THI], i16)
            nc.sync.dma_start(out=dstl_hi[:], in_=dstl_hi_in[:])
            dinv_loc = cpool.tile([128, NG], f32)
            nc.sync.dma_start(out=dinv_loc[:], in_=dinv_loc_in[:])
            dinvb = cpool.tile([128, NLOCP], f32)
            nc.sync.dma_start(out=dinvb[:], in_=dinvb_in[:])
            Acat = cpool.tile([128, 3 * D], f32)
            nc.sync.dma_start(out=Acat[:], in_=Acat_in[:])
            Bcat = cpool.tile([128, 3 * NCLS], f32)
            nc.sync.dma_start(out=Bcat[:], in_=Bcat_in[:])
            b1_t = cpool.tile([128, 1], f32)
            nc.sync.dma_start(out=b1_t[:], in_=b1_in[:])
            b2_t = cpool.tile([128, 1], f32)
            nc.sync.dma_start(out=b2_t[:], in_=b2_in[:])

            # ---------- phase 0: T0 slice = dinv*x ; xT (feature-major x) ----------
            xT = bigpool.tile([128, NLOCP], f32, tag="big")
            for g in range(NG):
                xin = wpool.tile([P, D], f32, tag="xin")
                nc.sync.dma_start(out=xin[:], in_=x_in[g * P:(g + 1) * P, :])
                t0nm = wpool.tile([P, D], f32, tag="tnm")
                nc.scalar.activation(out=t0nm[:], in_=xin[:], func=Act.Copy,
                                     scale=dinv_loc[:, g:g + 1])
                nc.sync.dma_start(out=tslices[0][g * P:(g + 1) * P, :], in_=t0nm[:])
                trp = pmisc.tile([P, D], f32, tag="ptr")
                nc.tensor.transpose(out=trp[:], in_=xin[:], identity=ident[:])
                nc.vector.tensor_copy(out=xT[:, g * P:(g + 1) * P], in_=trp[:])

            ag_instrs = []
            ag = nc.gpsimd.collective_compute(
                "AllGather", Alu.bypass, replica_groups=rg,
                ins=[tslices[0][:]], outs=[tables[0][:]])
            ag_instrs.append(ag)

            u1T = bigpool.tile([128, NLOCP], f32, tag="big")
            hT = None
            pT = None

            def spmv_groups(table, ag_ins):
                """Generate (g, acc_psum) for all dst groups, gathering from
                `table` in chunks of CH groups."""
                first_gather = [True]
                for g0 in range(0, NG, CH):
                    ngc = min(CH, NG - g0)
                    glo = gpool.tile([128, CH * TLO * D], f32, tag="glo")
                    glo3 = glo[:].rearrange("p (t d) -> p t d", d=D)
                    gi = nc.gpsimd.dma_gather(
                        out_ap=glo3[:, :ngc * TLO, :],
                        in_ap=table[0:HALF, :],
                        idxs_ap=gidx_lo[:, g0 * TLO * 8:(g0 + ngc) * TLO * 8],
                        num_idxs=ngc * TLO * P,
                        num_idxs_reg=ngc * TLO * P,
                        elem_size=D, single_packet=False)
                    ghi = gpool.tile([128, CH * THI * D], f32, tag="ghi")
                    ghi3 = ghi[:].rearrange("p (t d) -> p t d", d=D)
                    gi2 = nc.gpsimd.dma_gather(
                        out_ap=ghi3[:, :ngc * THI, :],
                        in_ap=table[HALF:VPAD, :],
                        idxs_ap=gidx_hi[:, g0 * THI * 8:(g0 + ngc) * THI * 8],
                        num_idxs=ngc * THI * P,
                        num_idxs_reg=ngc * THI * P,
                        elem_size=D, single_packet=False)
                    if first_gather[0] and ag_ins is not None:
                        _add_dep_helper(gi.ins, ag_ins.ins, sync=True,
                                        reason="gather waits for allgather")
                        _add_dep_helper(gi2.ins, ag_ins.ins, sync=True,
                                        reason="gather waits for allgather")
                        first_gather[0] = False
                    for g in range(g0, g0 + ngc):
                        acc = pacc.tile([P, P], f32, tag="acc")
                        for t in range(TLO):
                            oh = wpool.tile([P, P], f32, tag="oh")
                            nc.vector.tensor_tensor(
                                out=oh[:],
                                in0=dstl_lo[:, g * TLO + t:g * TLO + t + 1].to_broadcast([P, P]),
                                in1=iota_t[:], op=Alu.is_equal)
                            nc.tensor.matmul(
                                out=acc[:], lhsT=glo3[:, (g - g0) * TLO + t, :],
                                rhs=oh[:], start=(t == 0), stop=False)
                        for t in range(THI):
                            oh = wpool.tile([P, P], f32, tag="oh")
                            nc.vector.tensor_tensor(
                                out=oh[:],
                                in0=dstl_hi[:, g * THI + t:g * THI + t + 1].to_broadcast([P, P]),
                                in1=iota_t[:], op=Alu.is_equal)
                            nc.tensor.matmul(
                                out=acc[:], lhsT=ghi3[:, (g - g0) * THI + t, :],
                                rhs=oh[:], start=False, stop=(t == THI - 1))
                        yield g, acc

            def build_table(src_fm_ap, g, dst_slice):
                """dst table rows g*128.. = transpose(dinv * src_fm)."""
                tfm = wpool.tile([P, P], f32, tag="tfm")
                nc.vector.tensor_mul(out=tfm[:], in0=src_fm_ap,
                                     in1=dinvb[:, g * P:(g + 1) * P])
                trp = pmisc.tile([P, P], f32, tag="ptr")
                nc.tensor.transpose(out=trp[:], in_=tfm[:], identity=ident[:])
                tnm = wpool.tile([P, P], f32, tag="tnm")
                nc.scalar.activation(out=tnm[:], in_=trp[:], func=Act.Copy)
                nc.sync.dma_start(out=dst_slice[g * P:(g + 1) * P, :], in_=tnm[:])

            # ---------- phase 1: u1 = dinv*segsum(T0) ; build T1 ----------
            for g, acc in spmv_groups(tables[0], ag_instrs[0]):
                cols = slice(g * P, (g + 1) * P)
                nc.vector.tensor_mul(out=u1T[:, cols], in0=acc[:], in1=dinvb[:, cols])
                build_table(u1T[:, cols], g, tslices[1])
            ag = nc.gpsimd.collective_compute(
                "AllGather", Alu.bypass, replica_groups=rg,
                ins=[tslices[1][:]], outs=[tables[1][:]])
            ag_instrs.append(ag)

            # ---------- phase 2: v ; out1 = x@A0+u1@A1+v@A2 ; h=relu ; T2 ----------
            hT = bigpool.tile([128, NLOCP], f32, tag="big")
            for g, acc in spmv_groups(tables[1], ag_instrs[1]):
                cols = slice(g * P, (g + 1) * P)
                v = wpool.tile([P, P], f32, tag="w")
                nc.vector.tensor_mul(out=v[:], in0=acc[:], in1=dinvb[:, cols])
                o1 = pmisc.tile([P, P], f32, tag="pout")
                nc.tensor.matmul(out=o1[:], lhsT=Acat[:, 0:D], rhs=xT[:, cols],
                                 start=True, stop=False)
                nc.tensor.matmul(out=o1[:], lhsT=Acat[:, D:2 * D], rhs=u1T[:, cols],
                                 start=False, stop=False)
                nc.tensor.matmul(out=o1[:], lhsT=Acat[:, 2 * D:3 * D], rhs=v[:],
                                 start=False, stop=True)
                nc.scalar.activation(out=hT[:, cols], in_=o1[:], func=Act.Relu,
                                     bias=b1_t[:, 0:1])
                build_table(hT[:, cols], g, tslices[2])
            ag = nc.gpsimd.collective_compute(
                "AllGather", Alu.bypass, replica_groups=rg,
                ins=[tslices[2][:]], outs=[tables[2][:]])
            ag_instrs.append(ag)

            # ---------- phase 3: p = dinv*segsum(T2) ; build T3 ----------
            pT = bigpool.tile([128, NLOCP], f32, tag="big")
            for g, acc in spmv_groups(tables[2], ag_instrs[2]):
                cols = slice(g * P, (g + 1) * P)
                nc.vector.tensor_mul(out=pT[:, cols], in0=acc[:], in1=dinvb[:, cols])
                build_table(pT[:, cols], g, tslices[3])
            ag = nc.gpsimd.collective_compute(
                "AllGather", Alu.bypass, replica_groups=rg,
                ins=[tslices[3][:]], outs=[tables[3][:]])
            ag_instrs.append(ag)

            # ---------- phase 4: q ; out2 ; log_softmax ; write out ----------
            for g, acc in spmv_groups(tables[3], ag_instrs[3]):
                cols = slice(g * P, (g + 1) * P)
                q = wpool.tile([P, P], f32, tag="w")
                nc.vector.tensor_mul(out=q[:], in0=acc[:], in1=dinvb[:, cols])
                o2 = pmisc.tile([P, P], f32, tag="pout")
                nc.tensor.matmul(out=o2[:NCLS, :], lhsT=Bcat[:, 0:NCLS],
                                 rhs=hT[:, cols], start=True, stop=False)
                nc.tensor.matmul(out=o2[:NCLS, :], lhsT=Bcat[:, NCLS:2 * NCLS],
                                 rhs=pT[:, cols], start=False, stop=False)
                nc.tensor.matmul(out=o2[:NCLS, :], lhsT=Bcat[:, 2 * NCLS:3 * NCLS],
                                 rhs=q[:], start=False, stop=True)
                lgT = wpool.tile([NCLS, P], f32, tag="lgT")
                nc.scalar.activation(out=lgT[:], in_=o2[:NCLS, :],
                                     func=Act.Identity, bias=b2_t[:NCLS, 0:1])
                lg = pmisc.tile([P, NCLS], f32, tag="plg")
                nc.tensor.transpose(out=lg[:], in_=lgT[:], identity=ident[:NCLS, :NCLS])
                # log_softmax over free dim (classes)
                m = wpool.tile([P, 1], f32, tag="m")
                nc.vector.reduce_max(m[:], lg[:], axis=mybir.AxisListType.X)
                negm = wpool.tile([P, 1], f32, tag="negm")
                nc.vector.tensor_scalar_mul(negm[:], m[:], -1.0)
                e_t = wpool.tile([P, NCLS], f32, tag="e")
                s_t = wpool.tile([P, 1], f32, tag="s")
                nc.scalar.activation(out=e_t[:], in_=lg[:], func=Act.Exp,
                                     bias=negm[:, 0:1], accum_out=s_t[:])
                ls = wpool.tile([P, 1], f32, tag="ls")
                nc.scalar.activation(out=ls[:], in_=s_t[:], func=Act.Ln)
                fin = wpool.tile([P, NCLS], f32, tag="fin")
                nc.vector.tensor_scalar(
                    out=fin[:], in0=lg[:], scalar1=m[:, 0:1], scalar2=ls[:, 0:1],
                    op0=Alu.subtract, op1=Alu.subtract)
                rows = min(P, NLOC - g * P)
                nc.sync.dma_start(out=out_par[g * P:g * P + rows, :],
                                  in_=fin[:rows, :])

    nc.finalize()
    return nc


def _host_prep(x, edge_index, W1, b1, W2, b2):
    x = np.asarray(x, dtype=np.float32)
    ei = np.asarray(edge_index)
    W1 = np.asarray(W1, dtype=np.float32)
    b1 = np.asarray(b1, dtype=np.float32)
    W2 = np.asarray(W2, dtype=np.float32)
    b2 = np.asarray(b2, dtype=np.float32)
    src = ei[0].astype(np.int64)
    dst = ei[1].astype(np.int64)

    deg = np.bincount(src, minlength=N_NODES).astype(np.float32)
    dinv = np.where(deg > 0, 1.0 / np.sqrt(np.maximum(deg, 1e-12)), 0.0).astype(np.float32)

    # padded-table row of each node
    srow_all = (src // NLOC) * NLOCP + (src % NLOC)

    # per-core bucketing
    core = dst // NLOC
    ed = dst - core * NLOC
    grp = ed // P
    dl = (ed % P).astype(np.int16)
    is_hi = srow_all >= HALF

    per_core = []
    max_lo = 0
    max_hi = 0
    for c in range(NCORES):
        m = core == c
        g_c = grp[m]
        key = g_c * 2 + is_hi[m]
        order = np.argsort(key, kind='stable')
        ks = key[order]
        srow_s = srow_all[m][order]
        dl_s = dl[m][order]
        counts = np.bincount(ks, minlength=2 * NG)
        starts = np.concatenate([[0], np.cumsum(counts)[:-1]])
        within = np.arange(len(ks)) - starts[ks]
        per_core.append((ks, srow_s, dl_s, within, counts))
        max_lo = max(max_lo, counts[0::2].max())
        max_hi = max(max_hi, counts[1::2].max())

    TLO = max(1, int(-(-max_lo // P)))
    THI = max(1, int(-(-max_hi // P)))

    A = np.stack([W1[0] - W1[2], -W1[1], 2.0 * W1[2]])   # [3,128,128]
    B = np.stack([W2[0] - W2[2], -W2[1], 2.0 * W2[2]])   # [3,128,40]
    Acat = np.concatenate([A[0], A[1], A[2]], axis=1).astype(np.float32)
    Bcat = np.concatenate([B[0], B[1], B[2]], axis=1).astype(np.float32)
    b1_col = np.zeros((128, 1), np.float32)
    b1_col[:, 0] = b1
    b2_col = np.zeros((128, 1), np.float32)
    b2_col[:NCLS, 0] = b2

    def wrap_idx(a):  # flat multiple of 128 -> [128, len/16] int16
        w = a.reshape(-1, 16).T.astype(np.int16)
        return np.ascontiguousarray(np.tile(w, (8, 1)))

    def wrap_dstl(a):  # flat multiple of 128 -> [128, len/128] int16
        return np.ascontiguousarray(a.reshape(-1, P).T.astype(np.int16))

    in_maps = []
    for c in range(NCORES):
        ks, srow_s, dl_s, within, counts = per_core[c]
        glo = np.zeros((NG, TLO * P), np.int64)
        ghi = np.zeros((NG, THI * P), np.int64)
        dlo = np.full((NG, TLO * P), -1, np.int16)
        dhi = np.full((NG, THI * P), -1, np.int16)
        lo_m = (ks % 2) == 0
        gg = ks // 2
        glo[gg[lo_m], within[lo_m]] = srow_s[lo_m]
        dlo[gg[lo_m], within[lo_m]] = dl_s[lo_m]
        hi_m = ~lo_m
        ghi[gg[hi_m], within[hi_m]] = srow_s[hi_m] - HALF
        dhi[gg[hi_m], within[hi_m]] = dl_s[hi_m]

        x_c = np.zeros((NLOCP, D), np.float32)
        x_c[:NLOC] = x[c * NLOC:(c + 1) * NLOC]
        dinv_c = np.zeros(NLOCP, np.float32)
        dinv_c[:NLOC] = dinv[c * NLOC:(c + 1) * NLOC]
        dinv_loc = np.ascontiguousarray(dinv_c.reshape(NG, P).T)      # [128, NG]
        dinvb = np.ascontiguousarray(
            np.broadcast_to(dinv_c[None, :], (128, NLOCP))).astype(np.float32)

        in_maps.append({
            "x": x_c,
            "gidx_lo": wrap_idx(glo.reshape(-1)),
            "gidx_hi": wrap_idx(ghi.reshape(-1)),
            "dstl_lo": wrap_dstl(dlo.reshape(-1)),
            "dstl_hi": wrap_dstl(dhi.reshape(-1)),
            "dinv_loc": dinv_loc,
            "dinvb": dinvb,
            "Acat": Acat,
            "Bcat": Bcat,
            "b1": b1_col,
            "b2": b2_col,
        })
    return in_maps, TLO, THI


def kernel(x, edge_index, W1, b1, W2, b2, _trace=False, _tmpdir=None):
    from concourse.bass_utils import run_bass_kernel_spmd

    in_maps, TLO, THI = _host_prep(x, edge_index, W1, b1, W2, b2)
    key = (TLO, THI)
    if key not in _prog_cache:
        _prog_cache[key] = _build_program(TLO, THI)
    nc = _prog_cache[key]

    res = run_bass_kernel_spmd(nc, in_maps, list(range(NCORES)),
                               trace=_trace, tmpdir=_tmpdir)
    out = np.concatenate([res.results[c]["out"] for c in range(NCORES)], axis=0)
    kernel._last_results = res
    return out

